# revision 9
# baseline (speedup 1.0000x reference)
"""Multi-head attention (SEQ=4096, EMBED=1024, 16 heads, Dh=64) on 8 TRN2
NeuronCores, head-parallel: 2 heads per core, Wo row-sharded so each core
emits a partial output [SEQ, EMBED]; the host sums the 8 partials (+bo).

v2 pipeline (vs. the 403us baseline, which was ScalarE-exp-bound at 293us):
  - exp split across TWO engines: ScalarE does exact table exp for most
    c-chunks; the DVE does a Schraudolph exp for the rest (one tensor_scalar
    fp32->int32 convert: i = (int)(x*2^23/ln2 + B); the int32 bit pattern IS
    exp(x) to +-3%, bitcast to f32r for the matmul). Softmax cancels the
    correlated error (measured final rel err 4.9e-3 even at 100% approx).
  - attention value path in f32r (vP, pT, xT) so both engines' exp output
    feeds the same matmuls; scores/projections stay bf16.
  - output projection: the two heads' K=64 matmuls run CONCURRENTLY via
    row-tiling (tile_position (0,0)/(64,0)), xT holds both heads stacked
    [128, SEQ]; per-head 1/D normalize fused into 2 DVE/ACT ops
    (tensor_scalar_mul + affine_then_add) instead of 3.
"""

import os
import sys

sys.path.insert(0, "/opt/trn_rl_repo")

import numpy as np

SEQ = 4096
EMBED = 1024
HEADS = 16
HD = 64
NCORES = 8
HPC = HEADS // NCORES  # 2 heads per core
EC = EMBED // 128  # 8 e-chunks
SUP = 512  # s-super size
NSUP = SEQ // SUP  # 8
TC = SEQ // 128  # 32 t-chunks
JS = SUP // 128  # 4 s-tiles per super

# Schraudolph exp in bf16: bitcast(int16(x * 2^7/ln2 + (127 - C)*2^7)) ~= e^x
# (bf16 = top 16 bits of fp32: 8-bit exponent + 7-bit mantissa)
K_SCH = float(np.float32(2**7 / np.log(2)))
B_SCH = float(np.float32((127.0 - 0.0430) * 2**7))

# knobs: how many of the 32 c-chunks per super the DVE exps (rest: ScalarE),
# and the fraction of output-units whose normalize-mul runs on ScalarE.
N_DVE_EXP = int(os.environ.get("K_DVE_EXP", "13"))
ACT_MUL_MOD = int(os.environ.get("K_ACT_MUL_MOD", "1"))  # every k-th unit

LAST = None  # BassKernelResults of the most recent run (read by test.py)
_CACHE = {}


def _build():
    import concourse.bacc as bacc
    import concourse.tile as tile
    from concourse import mybir

    f32 = mybir.dt.float32

    nc = bacc.Bacc("TRN2", debug=False, enable_asserts=False, num_devices=NCORES)

    bf16 = mybir.dt.bfloat16
    wqkv = nc.dram_tensor("w_qkv", [3, 128, EC, 128], bf16, kind="ExternalInput").ap()
    bqk = nc.dram_tensor("b_qk", [2, 128], f32, kind="ExternalInput").ap()
    bv = nc.dram_tensor("b_v", [128], f32, kind="ExternalInput").ap()
    wo = nc.dram_tensor("w_o", [128, EMBED], bf16, kind="ExternalInput").ap()
    ident = nc.dram_tensor("ident", [128, 128], f32, kind="ExternalInput").ap()
    outp = nc.dram_tensor("out_p", [SEQ, EMBED], f32, kind="ExternalOutput").ap()
    hidB = nc.dram_tensor(
        "hidden_bf16", [SEQ, EMBED], mybir.dt.bfloat16, kind="ExternalInput"
    ).ap()
    with tile.TileContext(nc) as tc:
        _emit(tc, mybir, hidB, wqkv, bqk, bv, wo, ident, outp)

    nc.compile()
    return nc


def _emit(tc, mybir, hidB, wqkv, bqk, bv, wo, ident, outp):
    import concourse.bass as bass

    nc = tc.nc
    ts = bass.ts
    f32 = mybir.dt.float32
    f32r = mybir.dt.float32r
    i16 = mybir.dt.int16
    bf16 = mybir.dt.bfloat16
    Exp = mybir.ActivationFunctionType.Exp
    AOT = mybir.AluOpType

    def _mm(ap):
        if ap.dtype == f32:
            return ap.bitcast(f32r)
        return ap

    import contextlib

    st_ = contextlib.ExitStack()
    persist = st_.enter_context(tc.tile_pool(name="persist", bufs=1))
    qT = persist.tile([128, SEQ], bf16, tag="qT")
    kT = persist.tile([128, SEQ], bf16, tag="kT")
    vP = persist.tile([128, TC, 2 * (HD + 1)], bf16, tag="vP")
    # both heads' attn^T stacked: rows 0-63 = h0, 64-127 = h1
    xT = persist.tile([128, SEQ], bf16, tag="xT")
    hTa = persist.tile([128, EC, SEQ], bf16, tag="hTa")  # hidden^T, all chunks
    wq_sb = persist.tile([128, EC, 128], bf16, tag="wq")
    wk_sb = persist.tile([128, EC, 128], bf16, tag="wk")
    wv_sb = persist.tile([128, EC, 128], bf16, tag="wv")
    wo_sb = persist.tile([128, EMBED], bf16, tag="wo")
    id_sb = persist.tile([128, 128], f32, tag="ident")
    bq_sb = persist.tile([128, 1], f32, tag="bq")
    bk_sb = persist.tile([128, 1], f32, tag="bk")
    bv_sb = persist.tile([128, 1], f32, tag="bv")
    idb_sb = persist.tile([128, 128], bf16, tag="idb")

    nc.sync.dma_start(out=id_sb, in_=ident)
    nc.vector.tensor_copy(out=idb_sb, in_=id_sb)
    nc.sync.dma_start(out=bq_sb, in_=bqk[0:1, :].rearrange("a p -> p a"))
    nc.sync.dma_start(out=bk_sb, in_=bqk[1:2, :].rearrange("a p -> p a"))
    bv_col = bass.AP(tensor=bv.tensor, offset=bv.offset, ap=[[1, 128], [1, 1]])
    nc.sync.dma_start(out=bv_sb, in_=bv_col)
    ones_sb = persist.tile([128, 1], f32, tag="ones")
    nc.vector.memset(ones_sb, 1.0)
    vP_ones = vP.rearrange("p c (h e) -> p c h e", h=2)[:, :, :, HD : HD + 1]
    ones_b = bass.AP(
        tensor=ones_sb.tensor, offset=ones_sb.offset,
        ap=[ones_sb.ap[0], [0, TC], [0, 2], [0, 1]],
    )
    nc.vector.tensor_copy(out=vP_ones, in_=ones_b)

    def ht_block(b, split=False):
        # hidden^T for s-block b via DMA xbar transpose (2-byte dtype).
        # DMA transpose runs only on the SP/Activation queues; split the
        # startup blocks across both so they land in ~5us instead of ~10us.
        for c in range(EC):
            eng = nc.scalar if (split and c % 2 == 1) else nc.sync
            eng.dma_start(
                out=hTa[:, c, ts(b, SUP)],
                in_=hidB[ts(b, SUP), ts(c, 128)],
                transpose=True,
            )

    ht_block(0, split=True)
    ht_block(1, split=True)
    for i, w_sb in enumerate((wq_sb, wk_sb, wv_sb)):
        nc.gpsimd.dma_start(out=w_sb, in_=wqkv[i])
    nc.gpsimd.dma_start(out=wo_sb, in_=wo)
    pT_p = st_.enter_context(tc.tile_pool(name="pT", bufs=6))
    vT_p = st_.enter_context(tc.tile_pool(name="vT", bufs=2))
    dtmp_p = st_.enter_context(tc.tile_pool(name="dtmp", bufs=2))
    rD_p = st_.enter_context(tc.tile_pool(name="rD", bufs=2))
    t0_p = st_.enter_context(tc.tile_pool(name="t0", bufs=4))
    sc_ps_p = st_.enter_context(tc.tile_pool(name="ps_sc", bufs=2, space="PSUM"))
    at_ps_p = st_.enter_context(tc.tile_pool(name="ps_at", bufs=1, space="PSUM"))
    aux_ps_p = st_.enter_context(tc.tile_pool(name="ps_aux", bufs=2, space="PSUM"))

    rd_of = {}
    at_of = {}
    d_of = {}

    # which c-chunks the DVE exps (evenly spread over the 32)
    dve_c = {
        c for c in range(TC)
        if ((c + 1) * N_DVE_EXP) // TC > (c * N_DVE_EXP) // TC
    }

    def q_proj(sup):
        q_ps = aux_ps_p.tile([128, SUP], f32, tag="aux", name=f"q_ps{sup}")
        for c in range(EC):
            nc.tensor.matmul(
                q_ps, wq_sb[:, c, :], hTa[:, c, ts(sup, SUP)],
                start=(c == 0), stop=(c == EC - 1),
            )
        nc.vector.tensor_scalar(
            out=qT[:, ts(sup, SUP)], in0=q_ps,
            scalar1=bq_sb, scalar2=None, op0=AOT.add,
        )

    def k_part(b):
        k_ps = aux_ps_p.tile([128, SUP], f32, tag="aux", name=f"k_ps{b}")
        for c in range(EC):
            nc.tensor.matmul(
                k_ps, wk_sb[:, c, :], hTa[:, c, ts(b, SUP)],
                start=(c == 0), stop=(c == EC - 1),
            )
        nc.vector.tensor_scalar(
            out=kT[:, ts(b, SUP)], in0=k_ps,
            scalar1=bk_sb, scalar2=None, op0=AOT.add,
        )

    def v_part(b):
        vT_ps = aux_ps_p.tile([128, SUP], f32, tag="aux", name=f"vT_ps{b}")
        for c in range(EC):
            nc.tensor.matmul(
                vT_ps, wv_sb[:, c, :], hTa[:, c, ts(b, SUP)],
                start=(c == 0), stop=(c == EC - 1),
            )
        vT_sb = vT_p.tile([128, SUP], bf16, tag="vT", name=f"vT{b}")
        nc.vector.tensor_scalar(
            out=vT_sb, in0=vT_ps, scalar1=bv_sb, scalar2=None, op0=AOT.add
        )
        tp_ps = aux_ps_p.tile([128, JS, 128], bf16, tag="aux", name=f"tp_ps{b}")
        for j in range(JS):
            nc.tensor.transpose(tp_ps[:, j, :], vT_sb[:, ts(j, 128)], idb_sb)
        for j in range(JS):
            t_idx = JS * b + j
            dst = vP[:, t_idx, :].rearrange("p (h e) -> p h e", h=2)[:, :, 0:HD]
            nc.vector.tensor_copy(
                out=dst,
                in_=tp_ps[:, j, :].rearrange("p (h d) -> p h d", h=2),
            )

    pT_of = {}

    def sc_exp(sup, c):
        sc_ps = sc_ps_p.tile([128, 2 * SUP], f32, tag="sc", name=f"sc{sup}_{c}")
        for h in range(HPC):
            nc.tensor.matmul(
                sc_ps[:, ts(h, SUP)],
                kT[ts(h, HD), ts(c, 128)],
                qT[ts(h, HD), ts(sup, SUP)],
                start=True, stop=True,
                tile_position=(h * HD, 0),
            )
        pT = pT_p.tile([128, 2 * SUP], bf16, tag="pT", name=f"pT{sup}_{c}")
        if c in dve_c:
            # Schraudolph exp on the DVE: int32 convert of K*x+B
            nc.vector.tensor_scalar(
                out=pT.bitcast(i16), in0=sc_ps,
                scalar1=K_SCH, scalar2=B_SCH, op0=AOT.mult, op1=AOT.add,
            )
        else:
            nc.scalar.activation(out=pT, in_=sc_ps, func=Exp)
        pT_of[(sup, c)] = pT

    def at_mms(sup, c):
        pT = pT_of.pop((sup, c))
        for h in range(HPC):
            nc.tensor.matmul(
                at_of[sup][h],
                vP[:, c, ts(h, HD + 1)],
                pT[:, ts(h, SUP)],
                start=(c == 0), stop=(c == TC - 1),
            )

    def drain(sup):
        dts = [
            dtmp_p.tile([1, SUP], f32, tag=f"d{h}", name=f"d{sup}_{h}")
            for h in range(HPC)
        ]
        d_of[sup] = dts
        ssl = ts(sup, SUP)
        for h in range(HPC):
            nc.vector.tensor_copy(
                out=xT[ts(h, HD), ssl], in_=at_of[sup][h][0:HD, :]
            )
            nc.vector.tensor_copy(out=dts[h], in_=at_of[sup][h][HD : HD + 1, :])

    def c_head(sup):
        # denominators -> s-partitioned reciprocals
        dT_ps = sc_ps_p.tile([128, HPC * JS], f32, tag="sc", name=f"dT{sup}")
        for h in range(HPC):
            for j in range(JS):
                nc.tensor.transpose(
                    dT_ps[:, h * JS + j : h * JS + j + 1],
                    d_of[sup][h][:, ts(j, 128)],
                    id_sb[0:1, 0:1],
                )
        rD = rD_p.tile([128, HPC, JS], f32, tag="rD", name=f"rD{sup}")
        nc.vector.reciprocal(out=rD.rearrange("p h j -> p (h j)"), in_=dT_ps)
        rd_of[sup] = rD

    def c_unit(sup, j, eh, alt_pool=None):
        st_i = JS * sup + j
        rD = rd_of[sup]
        pool, tag = (
            (alt_pool, "sc") if alt_pool is not None else (aux_ps_p, "aux")
        )
        o_ps = [
            pool.tile([128, SUP], f32, tag=tag, name=f"o{st_i}_{eh}_{h}")
            for h in range(HPC)
        ]
        # concurrent row-tiled pair: h0 rows 0-63, h1 rows 64-127
        nc.tensor.matmul(
            o_ps[0], xT[0:HD, ts(st_i, 128)],
            wo_sb[0:HD, ts(eh, SUP)],
            start=True, stop=True, tile_position=(0, 0),
        )
        nc.tensor.matmul(
            o_ps[1], xT[HD:128, ts(st_i, 128)],
            wo_sb[HD:128, ts(eh, SUP)],
            start=True, stop=True, tile_position=(HD, 0),
        )
        t0 = t0_p.tile([128, SUP], f32, tag="t0", name=f"t0_{st_i}_{eh}")
        t1 = t0_p.tile([128, SUP], f32, tag="t1", name=f"t1_{st_i}_{eh}")
        nc.scalar.mul(t0, o_ps[0], rD[:, 0, j : j + 1])
        nc.vector.tensor_scalar_mul(out=t1, in0=o_ps[1], scalar1=rD[:, 1, j : j + 1])
        osl = outp[ts(st_i, 128), ts(eh, SUP)]
        # both on the gpsimd (SWDGE) queue: in-order, so the accumulate
        # lands after the base write; CCE in the SDMA datapath does the add
        nc.gpsimd.dma_start(out=osl, in_=t0)
        nc.gpsimd.dma_start(out=osl, in_=t1, accum_op=AOT.add)

    def c_tail(sup, slot, alt_every_other=False):
        # slot 0: head (D/recip); slots 1..8: the 8 (j, eh) units
        if slot == 0:
            c_head(sup)
        else:
            u = slot - 1
            j, eh = divmod(u, EMBED // SUP)
            alt = sc_ps_p if (alt_every_other and u % 2 == 1) else None
            c_unit(sup, j, eh, alt_pool=alt)

    # ---- phase A interleaved with super 0 (lag-one chunk groups) ---------
    at_of[0] = [
        at_ps_p.tile([HD + 1, SUP], f32, tag=f"at{h}", name=f"at0_{h}")
        for h in range(HPC)
    ]
    k_part(0)
    q_proj(0)
    v_part(0)
    for b in range(1, NSUP):
        if b + 1 < NSUP:
            ht_block(b + 1)
        for c in range(JS * (b - 1), JS * b):
            sc_exp(0, c)
            if c > 0:
                at_mms(0, c - 1)
        k_part(b)
        v_part(b)
    for c in range(JS * (NSUP - 1), TC):
        sc_exp(0, c)
        at_mms(0, c - 1)
    q_proj(1)

    # ---- supers 1..7 with trailing C(sup-1), q_proj(sup+1) in-stream -----
    for sup in range(1, NSUP):
        slot = 0
        for c in range(TC):
            sc_exp(sup, c)
            if c == 0:
                at_mms(sup - 1, TC - 1)
                drain(sup - 1)
                at_of[sup] = [
                    at_ps_p.tile(
                        [HD + 1, SUP], f32, tag=f"at{h}", name=f"at{sup}_{h}"
                    )
                    for h in range(HPC)
                ]
            else:
                at_mms(sup, c - 1)
            if c % 3 == 2 and slot < 9:
                c_tail(sup - 1, slot)
                slot += 1
            if c == 29 and sup + 1 < NSUP:
                q_proj(sup + 1)
        while slot < 9:
            c_tail(sup - 1, slot)
            slot += 1
    at_mms(NSUP - 1, TC - 1)
    drain(NSUP - 1)
    for slot in range(9):
        c_tail(NSUP - 1, slot, alt_every_other=True)

    st_.close()


def _shards(inputs):
    """Host-side prep: per-core input dicts (head-parallel, Wo row-shard)."""
    import ml_dtypes

    hs = np.ascontiguousarray(np.asarray(inputs["hidden_state"], np.float32))
    Wq = np.asarray(inputs["Wq"], np.float32) * 0.125  # fold 1/sqrt(64); exact
    bq = np.asarray(inputs["bq"], np.float32) * 0.125
    Wk = np.asarray(inputs["Wk"], np.float32)
    bk = np.asarray(inputs["bk"], np.float32)
    Wv = np.asarray(inputs["Wv"], np.float32)
    bv = np.asarray(inputs["bv"], np.float32)
    Wo = np.asarray(inputs["Wo"], np.float32)
    ident = np.eye(128, dtype=np.float32)
    hs_bf16 = np.ascontiguousarray(hs.astype(ml_dtypes.bfloat16))

    in_maps = []
    for c in range(NCORES):
        h0 = HPC * c
        # [H,E,Dh] head-pair -> [E, 2*Dh] -> [EC, 128, 128]
        def _w(W):
            w = np.transpose(W[h0 : h0 + HPC], (1, 0, 2)).reshape(EMBED, 128)
            # [E, 128] -> [128part(e%128), EC, 128d] matching hTa chunking
            w = w.reshape(EC, 128, 128).transpose(1, 0, 2)
            return np.ascontiguousarray(w.astype(ml_dtypes.bfloat16))

        w_qkv = np.stack([_w(Wq), _w(Wk), _w(Wv)])
        b_qk = np.stack(
            [bq[h0 : h0 + HPC].reshape(128), bk[h0 : h0 + HPC].reshape(128)]
        )
        b_v = np.ascontiguousarray(bv[h0 : h0 + HPC].reshape(128))
        # Wo rows for this core's heads, natural: rows 0-63 = h0, 64-127 = h1
        w_o = np.ascontiguousarray(
            Wo[128 * c : 128 * (c + 1)].astype(ml_dtypes.bfloat16)
        )
        in_maps.append(
            {
                "hidden_bf16": hs_bf16,
                "w_qkv": np.ascontiguousarray(w_qkv),
                "b_qk": np.ascontiguousarray(b_qk),
                "b_v": b_v,
                "w_o": w_o,
                "ident": ident,
            }
        )
    return in_maps


def kernel(**inputs):
    global LAST
    from concourse import bass_utils

    trace = bool(int(os.environ.get("K_TRACE", "0")))
    if trace:
        _install_ntff_shim()

    key = (N_DVE_EXP, ACT_MUL_MOD)
    if key not in _CACHE:
        _CACHE[key] = _build()
    nc = _CACHE[key]

    in_maps = _shards(inputs)
    res = bass_utils.run_bass_kernel_spmd(
        nc, in_maps, core_ids=list(range(NCORES)), trace=trace
    )
    LAST = res

    out = np.zeros((SEQ, EMBED), np.float64)
    for c in range(NCORES):
        out += res.results[c]["out_p"].astype(np.float64)
    out += np.asarray(inputs["bo"], np.float32).astype(np.float64)
    return out.astype(np.float32)


def _install_ntff_shim():
    """antenv.axon_hooks is absent from this image; recreate it so
    run_bass_kernel_spmd(trace=True) can reach the NTFF profiling hook."""
    import types

    if "antenv.axon_hooks" in sys.modules:
        return
    try:
        if "/root/.axon_site" not in sys.path:
            sys.path.insert(0, "/root/.axon_site")
        from trn_agent_boot.trn_boot import _ntff_profile_via_ctypes

        hook = _ntff_profile_via_ctypes("/opt/axon/libaxon_pjrt.so")
    except Exception:
        hook = None
    mod = types.ModuleType("antenv.axon_hooks")
    mod._hook = hook
    mod.get_axon_ntff_profile_hook = lambda: mod._hook
    mod.set_axon_ntff_profile_hook = lambda h: setattr(mod, "_hook", h)
    sys.modules["antenv.axon_hooks"] = mod


# revision 13
# speedup vs baseline: 1.0445x; 1.0445x over previous
"""Multi-head attention (SEQ=4096, EMBED=1024, 16 heads, Dh=64) on 8 TRN2
NeuronCores, head-parallel: 2 heads per core, Wo row-sharded so each core
emits a partial output [SEQ, EMBED]; the host sums the 8 partials (+bo).

v2 pipeline (vs. the 403us baseline, which was ScalarE-exp-bound at 293us):
  - exp split across TWO engines: ScalarE does exact table exp for most
    c-chunks; the DVE does a Schraudolph exp for the rest (one tensor_scalar
    fp32->int32 convert: i = (int)(x*2^23/ln2 + B); the int32 bit pattern IS
    exp(x) to +-3%, bitcast to f32r for the matmul). Softmax cancels the
    correlated error (measured final rel err 4.9e-3 even at 100% approx).
  - attention value path in f32r (vP, pT, xT) so both engines' exp output
    feeds the same matmuls; scores/projections stay bf16.
  - output projection: the two heads' K=64 matmuls run CONCURRENTLY via
    row-tiling (tile_position (0,0)/(64,0)), xT holds both heads stacked
    [128, SEQ]; per-head 1/D normalize fused into 2 DVE/ACT ops
    (tensor_scalar_mul + affine_then_add) instead of 3.
"""

import os
import sys

sys.path.insert(0, "/opt/trn_rl_repo")

import numpy as np

SEQ = 4096
EMBED = 1024
HEADS = 16
HD = 64
NCORES = 8
HPC = HEADS // NCORES  # 2 heads per core
EC = EMBED // 128  # 8 e-chunks
SUP = 512  # s-super size
NSUP = SEQ // SUP  # 8
TC = SEQ // 128  # 32 t-chunks
JS = SUP // 128  # 4 s-tiles per super

# Schraudolph exp in bf16: bitcast(int16(x * 2^7/ln2 + (127 - C)*2^7)) ~= e^x
# (bf16 = top 16 bits of fp32: 8-bit exponent + 7-bit mantissa)
K_SCH = float(np.float32(2**7 / np.log(2)))
B_SCH = float(np.float32((127.0 - 0.0430) * 2**7))

# knobs: how many of the 32 c-chunks per super the DVE exps (rest: ScalarE),
# and the fraction of output-units whose normalize-mul runs on ScalarE.
N_DVE_EXP = int(os.environ.get("K_DVE_EXP", "14"))
ACT_MUL_MOD = int(os.environ.get("K_ACT_MUL_MOD", "1"))  # every k-th unit

LAST = None  # BassKernelResults of the most recent run (read by test.py)
_CACHE = {}


def _build():
    import concourse.bacc as bacc
    import concourse.tile as tile
    from concourse import mybir

    f32 = mybir.dt.float32

    nc = bacc.Bacc("TRN2", debug=False, enable_asserts=False, num_devices=NCORES)

    bf16 = mybir.dt.bfloat16
    wqkv = nc.dram_tensor("w_qkv", [3, 128, EC, 128], bf16, kind="ExternalInput").ap()
    bqk = nc.dram_tensor("b_qk", [2, 128], f32, kind="ExternalInput").ap()
    bv = nc.dram_tensor("b_v", [128], f32, kind="ExternalInput").ap()
    wo = nc.dram_tensor("w_o", [128, EMBED], bf16, kind="ExternalInput").ap()
    ident = nc.dram_tensor("ident", [128, 128], f32, kind="ExternalInput").ap()
    outp0 = nc.dram_tensor("out_p0", [SEQ, EMBED], f32, kind="ExternalOutput").ap()
    outp1 = nc.dram_tensor("out_p1", [SEQ, EMBED], f32, kind="ExternalOutput").ap()
    hidB = nc.dram_tensor(
        "hidden_bf16", [SEQ, EMBED], mybir.dt.bfloat16, kind="ExternalInput"
    ).ap()
    with tile.TileContext(nc) as tc:
        _emit(tc, mybir, hidB, wqkv, bqk, bv, wo, ident, outp0, outp1)

    nc.compile()
    return nc


def _emit(tc, mybir, hidB, wqkv, bqk, bv, wo, ident, outp0, outp1):
    import concourse.bass as bass

    nc = tc.nc
    ts = bass.ts
    f32 = mybir.dt.float32
    f32r = mybir.dt.float32r
    i16 = mybir.dt.int16
    bf16 = mybir.dt.bfloat16
    Exp = mybir.ActivationFunctionType.Exp
    AOT = mybir.AluOpType

    def _mm(ap):
        if ap.dtype == f32:
            return ap.bitcast(f32r)
        return ap

    import contextlib

    st_ = contextlib.ExitStack()
    persist = st_.enter_context(tc.tile_pool(name="persist", bufs=1))
    qT = persist.tile([128, SEQ], bf16, tag="qT")
    kT = persist.tile([128, SEQ], bf16, tag="kT")
    vP = persist.tile([128, TC, 2 * (HD + 1)], bf16, tag="vP")
    # both heads' attn^T stacked: rows 0-63 = h0, 64-127 = h1
    xT = persist.tile([128, SEQ], bf16, tag="xT")
    hTa = persist.tile([128, EC, SEQ], bf16, tag="hTa")  # hidden^T, all chunks
    wq_sb = persist.tile([128, EC, 128], bf16, tag="wq")
    wk_sb = persist.tile([128, EC, 128], bf16, tag="wk")
    wv_sb = persist.tile([128, EC, 128], bf16, tag="wv")
    wo_sb = persist.tile([128, EMBED], bf16, tag="wo")
    id_sb = persist.tile([128, 128], f32, tag="ident")
    bq_sb = persist.tile([128, 1], f32, tag="bq")
    bk_sb = persist.tile([128, 1], f32, tag="bk")
    bv_sb = persist.tile([128, 1], f32, tag="bv")
    idb_sb = persist.tile([128, 128], bf16, tag="idb")

    nc.sync.dma_start(out=id_sb, in_=ident)
    nc.vector.tensor_copy(out=idb_sb, in_=id_sb)
    nc.sync.dma_start(out=bq_sb, in_=bqk[0:1, :].rearrange("a p -> p a"))
    nc.sync.dma_start(out=bk_sb, in_=bqk[1:2, :].rearrange("a p -> p a"))
    bv_col = bass.AP(tensor=bv.tensor, offset=bv.offset, ap=[[1, 128], [1, 1]])
    nc.sync.dma_start(out=bv_sb, in_=bv_col)
    ones_sb = persist.tile([128, 1], f32, tag="ones")
    nc.vector.memset(ones_sb, 1.0)
    vP_ones = vP.rearrange("p c (h e) -> p c h e", h=2)[:, :, :, HD : HD + 1]
    ones_b = bass.AP(
        tensor=ones_sb.tensor, offset=ones_sb.offset,
        ap=[ones_sb.ap[0], [0, TC], [0, 2], [0, 1]],
    )
    nc.vector.tensor_copy(out=vP_ones, in_=ones_b)

    def ht_block(b, split=False):
        # hidden^T for s-block b via DMA xbar transpose (2-byte dtype).
        for c in range(EC):
            eng = nc.scalar if (split and c % 2 == 1) else nc.sync
            eng.dma_start(
                out=hTa[:, c, ts(b, SUP)],
                in_=hidB[ts(b, SUP), ts(c, 128)],
                transpose=True,
            )

    ht_block(0, split=True)
    ht_block(1, split=True)
    for i, w_sb in enumerate((wq_sb, wk_sb, wv_sb)):
        nc.gpsimd.dma_start(out=w_sb, in_=wqkv[i])
    nc.gpsimd.dma_start(out=wo_sb, in_=wo)
    pT_p = st_.enter_context(tc.tile_pool(name="pT", bufs=6))
    vT_p = st_.enter_context(tc.tile_pool(name="vT", bufs=2))
    dtmp_p = st_.enter_context(tc.tile_pool(name="dtmp", bufs=2))
    rD_p = st_.enter_context(tc.tile_pool(name="rD", bufs=2))
    t0_p = st_.enter_context(tc.tile_pool(name="t0", bufs=4))
    sc_ps_p = st_.enter_context(tc.tile_pool(name="ps_sc", bufs=2, space="PSUM"))
    at_ps_p = st_.enter_context(tc.tile_pool(name="ps_at", bufs=1, space="PSUM"))
    aux_ps_p = st_.enter_context(tc.tile_pool(name="ps_aux", bufs=2, space="PSUM"))

    rd_of = {}
    at_of = {}
    d_of = {}

    # which c-chunks the DVE exps (evenly spread over the 32)
    dve_c = {
        c for c in range(TC)
        if ((c + 1) * N_DVE_EXP) // TC > (c * N_DVE_EXP) // TC
    }

    def q_proj(sup):
        q_ps = aux_ps_p.tile([128, SUP], f32, tag="aux", name=f"q_ps{sup}")
        for c in range(EC):
            nc.tensor.matmul(
                q_ps, wq_sb[:, c, :], hTa[:, c, ts(sup, SUP)],
                start=(c == 0), stop=(c == EC - 1),
            )
        nc.vector.tensor_scalar(
            out=qT[:, ts(sup, SUP)], in0=q_ps,
            scalar1=bq_sb, scalar2=None, op0=AOT.add,
        )

    def k_part(b):
        k_ps = aux_ps_p.tile([128, SUP], f32, tag="aux", name=f"k_ps{b}")
        for c in range(EC):
            nc.tensor.matmul(
                k_ps, wk_sb[:, c, :], hTa[:, c, ts(b, SUP)],
                start=(c == 0), stop=(c == EC - 1),
            )
        nc.vector.tensor_scalar(
            out=kT[:, ts(b, SUP)], in0=k_ps,
            scalar1=bk_sb, scalar2=None, op0=AOT.add,
        )

    def v_part(b):
        vT_ps = aux_ps_p.tile([128, SUP], f32, tag="aux", name=f"vT_ps{b}")
        for c in range(EC):
            nc.tensor.matmul(
                vT_ps, wv_sb[:, c, :], hTa[:, c, ts(b, SUP)],
                start=(c == 0), stop=(c == EC - 1),
            )
        vT_sb = vT_p.tile([128, SUP], bf16, tag="vT", name=f"vT{b}")
        nc.vector.tensor_scalar(
            out=vT_sb, in0=vT_ps, scalar1=bv_sb, scalar2=None, op0=AOT.add
        )
        tp_ps = aux_ps_p.tile([128, JS, 128], bf16, tag="aux", name=f"tp_ps{b}")
        for j in range(JS):
            nc.tensor.transpose(tp_ps[:, j, :], vT_sb[:, ts(j, 128)], idb_sb)
        for j in range(JS):
            t_idx = JS * b + j
            dst = vP[:, t_idx, :].rearrange("p (h e) -> p h e", h=2)[:, :, 0:HD]
            nc.vector.tensor_copy(
                out=dst,
                in_=tp_ps[:, j, :].rearrange("p (h d) -> p h d", h=2),
            )

    pT_of = {}

    def sc_exp(sup, c):
        sc_ps = sc_ps_p.tile([128, 2 * SUP], f32, tag="sc", name=f"sc{sup}_{c}")
        for h in range(HPC):
            nc.tensor.matmul(
                sc_ps[:, ts(h, SUP)],
                kT[ts(h, HD), ts(c, 128)],
                qT[ts(h, HD), ts(sup, SUP)],
                start=True, stop=True,
                tile_position=(h * HD, 0),
            )
        pT = pT_p.tile([128, 2 * SUP], bf16, tag="pT", name=f"pT{sup}_{c}")
        if c in dve_c:
            # Schraudolph exp on the DVE: int32 convert of K*x+B
            nc.vector.tensor_scalar(
                out=pT.bitcast(i16), in0=sc_ps,
                scalar1=K_SCH, scalar2=B_SCH, op0=AOT.mult, op1=AOT.add,
            )
        else:
            nc.scalar.activation(out=pT, in_=sc_ps, func=Exp)
        pT_of[(sup, c)] = pT

    def at_mms(sup, c):
        pT = pT_of.pop((sup, c))
        for h in range(HPC):
            nc.tensor.matmul(
                at_of[sup][h],
                vP[:, c, ts(h, HD + 1)],
                pT[:, ts(h, SUP)],
                start=(c == 0), stop=(c == TC - 1),
            )

    def drain(sup):
        dts = [
            dtmp_p.tile([1, SUP], f32, tag=f"d{h}", name=f"d{sup}_{h}")
            for h in range(HPC)
        ]
        d_of[sup] = dts
        ssl = ts(sup, SUP)
        for h in range(HPC):
            nc.vector.tensor_copy(
                out=xT[ts(h, HD), ssl], in_=at_of[sup][h][0:HD, :]
            )
            nc.vector.tensor_copy(out=dts[h], in_=at_of[sup][h][HD : HD + 1, :])

    def c_head(sup):
        # denominators -> s-partitioned reciprocals
        dT_ps = sc_ps_p.tile([128, HPC * JS], f32, tag="sc", name=f"dT{sup}")
        for h in range(HPC):
            for j in range(JS):
                nc.tensor.transpose(
                    dT_ps[:, h * JS + j : h * JS + j + 1],
                    d_of[sup][h][:, ts(j, 128)],
                    id_sb[0:1, 0:1],
                )
        rD = rD_p.tile([128, HPC, JS], f32, tag="rD", name=f"rD{sup}")
        nc.vector.reciprocal(out=rD.rearrange("p h j -> p (h j)"), in_=dT_ps)
        rd_of[sup] = rD

    def c_unit(sup, j, eh, alt_pool=None):
        st_i = JS * sup + j
        rD = rd_of[sup]
        pool, tag = (
            (alt_pool, "sc") if alt_pool is not None else (aux_ps_p, "aux")
        )
        o_ps = [
            pool.tile([128, SUP], f32, tag=tag, name=f"o{st_i}_{eh}_{h}")
            for h in range(HPC)
        ]
        # concurrent row-tiled pair: h0 rows 0-63, h1 rows 64-127
        nc.tensor.matmul(
            o_ps[0], xT[0:HD, ts(st_i, 128)],
            wo_sb[0:HD, ts(eh, SUP)],
            start=True, stop=True, tile_position=(0, 0),
        )
        nc.tensor.matmul(
            o_ps[1], xT[HD:128, ts(st_i, 128)],
            wo_sb[HD:128, ts(eh, SUP)],
            start=True, stop=True, tile_position=(HD, 0),
        )
        t0 = t0_p.tile([128, SUP], f32, tag="t0", name=f"t0_{st_i}_{eh}")
        t1 = t0_p.tile([128, SUP], f32, tag="t1", name=f"t1_{st_i}_{eh}")
        nc.scalar.mul(t0, o_ps[0], rD[:, 0, j : j + 1])
        nc.vector.tensor_scalar_mul(out=t1, in0=o_ps[1], scalar1=rD[:, 1, j : j + 1])
        nc.sync.dma_start(out=outp0[ts(st_i, 128), ts(eh, SUP)], in_=t0)
        nc.sync.dma_start(out=outp1[ts(st_i, 128), ts(eh, SUP)], in_=t1)

    def c_tail(sup, slot, alt_every_other=False):
        # slot 0: head (D/recip); slots 1..8: the 8 (j, eh) units
        if slot == 0:
            c_head(sup)
        else:
            u = slot - 1
            j, eh = divmod(u, EMBED // SUP)
            alt = sc_ps_p if (alt_every_other and u % 2 == 1) else None
            c_unit(sup, j, eh, alt_pool=alt)

    # ---- phase A interleaved with super 0 (lag-one chunk groups) ---------
    at_of[0] = [
        at_ps_p.tile([HD + 1, SUP], f32, tag=f"at{h}", name=f"at0_{h}")
        for h in range(HPC)
    ]
    k_part(0)
    q_proj(0)
    v_part(0)
    for b in range(1, NSUP):
        if b + 1 < NSUP:
            ht_block(b + 1)
        for c in range(JS * (b - 1), JS * b):
            sc_exp(0, c)
            if c > 0:
                at_mms(0, c - 1)
        k_part(b)
        v_part(b)
    for c in range(JS * (NSUP - 1), TC):
        sc_exp(0, c)
        at_mms(0, c - 1)
    q_proj(1)

    # ---- supers 1..7 with trailing C(sup-1), q_proj(sup+1) in-stream -----
    for sup in range(1, NSUP):
        slot = 0
        for c in range(TC):
            sc_exp(sup, c)
            if c == 0:
                at_mms(sup - 1, TC - 1)
                drain(sup - 1)
                at_of[sup] = [
                    at_ps_p.tile(
                        [HD + 1, SUP], f32, tag=f"at{h}", name=f"at{sup}_{h}"
                    )
                    for h in range(HPC)
                ]
            else:
                at_mms(sup, c - 1)
            if c % 3 == 2 and slot < 9:
                c_tail(sup - 1, slot)
                slot += 1
            if c == 29 and sup + 1 < NSUP:
                q_proj(sup + 1)
        while slot < 9:
            c_tail(sup - 1, slot)
            slot += 1
    at_mms(NSUP - 1, TC - 1)
    drain(NSUP - 1)
    for slot in range(9):
        c_tail(NSUP - 1, slot, alt_every_other=True)

    st_.close()


def _shards(inputs):
    """Host-side prep: per-core input dicts (head-parallel, Wo row-shard)."""
    import ml_dtypes

    hs = np.ascontiguousarray(np.asarray(inputs["hidden_state"], np.float32))
    Wq = np.asarray(inputs["Wq"], np.float32) * 0.125  # fold 1/sqrt(64); exact
    bq = np.asarray(inputs["bq"], np.float32) * 0.125
    Wk = np.asarray(inputs["Wk"], np.float32)
    bk = np.asarray(inputs["bk"], np.float32)
    Wv = np.asarray(inputs["Wv"], np.float32)
    bv = np.asarray(inputs["bv"], np.float32)
    Wo = np.asarray(inputs["Wo"], np.float32)
    ident = np.eye(128, dtype=np.float32)
    hs_bf16 = np.ascontiguousarray(hs.astype(ml_dtypes.bfloat16))

    in_maps = []
    for c in range(NCORES):
        h0 = HPC * c
        # [H,E,Dh] head-pair -> [E, 2*Dh] -> [EC, 128, 128]
        def _w(W):
            w = np.transpose(W[h0 : h0 + HPC], (1, 0, 2)).reshape(EMBED, 128)
            # [E, 128] -> [128part(e%128), EC, 128d] matching hTa chunking
            w = w.reshape(EC, 128, 128).transpose(1, 0, 2)
            return np.ascontiguousarray(w.astype(ml_dtypes.bfloat16))

        w_qkv = np.stack([_w(Wq), _w(Wk), _w(Wv)])
        b_qk = np.stack(
            [bq[h0 : h0 + HPC].reshape(128), bk[h0 : h0 + HPC].reshape(128)]
        )
        b_v = np.ascontiguousarray(bv[h0 : h0 + HPC].reshape(128))
        # Wo rows for this core's heads, natural: rows 0-63 = h0, 64-127 = h1
        w_o = np.ascontiguousarray(
            Wo[128 * c : 128 * (c + 1)].astype(ml_dtypes.bfloat16)
        )
        in_maps.append(
            {
                "hidden_bf16": hs_bf16,
                "w_qkv": np.ascontiguousarray(w_qkv),
                "b_qk": np.ascontiguousarray(b_qk),
                "b_v": b_v,
                "w_o": w_o,
                "ident": ident,
            }
        )
    return in_maps


def kernel(**inputs):
    global LAST
    from concourse import bass_utils

    trace = bool(int(os.environ.get("K_TRACE", "0")))
    if trace:
        _install_ntff_shim()

    key = (N_DVE_EXP, ACT_MUL_MOD)
    if key not in _CACHE:
        _CACHE[key] = _build()
    nc = _CACHE[key]

    in_maps = _shards(inputs)
    res = bass_utils.run_bass_kernel_spmd(
        nc, in_maps, core_ids=list(range(NCORES)), trace=trace
    )
    LAST = res

    out = np.zeros((SEQ, EMBED), np.float64)
    for c in range(NCORES):
        out += res.results[c]["out_p0"].astype(np.float64)
        out += res.results[c]["out_p1"].astype(np.float64)
    out += np.asarray(inputs["bo"], np.float32).astype(np.float64)
    return out.astype(np.float32)


def _install_ntff_shim():
    """antenv.axon_hooks is absent from this image; recreate it so
    run_bass_kernel_spmd(trace=True) can reach the NTFF profiling hook."""
    import types

    if "antenv.axon_hooks" in sys.modules:
        return
    try:
        if "/root/.axon_site" not in sys.path:
            sys.path.insert(0, "/root/.axon_site")
        from trn_agent_boot.trn_boot import _ntff_profile_via_ctypes

        hook = _ntff_profile_via_ctypes("/opt/axon/libaxon_pjrt.so")
    except Exception:
        hook = None
    mod = types.ModuleType("antenv.axon_hooks")
    mod._hook = hook
    mod.get_axon_ntff_profile_hook = lambda: mod._hook
    mod.set_axon_ntff_profile_hook = lambda h: setattr(mod, "_hook", h)
    sys.modules["antenv.axon_hooks"] = mod
